# revision 2
# baseline (speedup 1.0000x reference)
"""GCLSTM (ChebConv-gated LSTM) Trainium2 kernel, 8-core SPMD, fp8 mega-prop.

Algorithm notes
---------------
reference computes, per timestep t (T=24) over N=5120 graph nodes:
    gate_g = X_t @ Ws[g] + cheb(H, thetas[g]) + biases      (4 gates)
    cheb(H, th) = H@th0 + (L@H)@th1 + (2L(LH) - H)@th2      (K=3 Chebyshev)
with L the scaled-normalized graph Laplacian (5120x5120, sparse, here
densified).  The Chebyshev basis (U = L@H, V = L^2@H) is shared by all 4
gates, so per step we need exactly ONE dense "mega-prop" [U|V] = [L;L^2]@H
plus the gate matmuls.  Folding:
    gate_g = X_t@Ws[g] + H@(th0-th2) + U@th1 + V@(2*th2) + b
so all gate work is a single [X;H;U;V] (1024) x Theta (1024x1024) matmul.

Sharding: nodes are split across 8 cores (640 each; edges connect
arbitrary nodes, so each core holds the full [L;L^2] column block for its
output rows, resident in SBUF).  The mega-prop contracts over ALL
5120 nodes, so the full H (node-major) is re-assembled every step with
two feature-half AllGathers.

Precision: the mega-prop runs in fp8 e4m3 with perf_mode=DoubleRow
(256-deep contraction, 2x PE throughput): ll2 holds [s_L*L | s_L2*L^2]
quantized to fp8, H is quantized to s_h*H fp8 (s_h=128) right after the
node-major transpose, and the AllGather moves fp8 bytes (half the wire
traffic).  U/V are descaled back to plain fp16 during the PSUM->SBUF
copy (scalar engine, scale from a tiny input tensor).  The gate matmul,
LSTM cell, and all theta weights stay fp16/fp32: simulation shows gate-
side fp8 compounds through the 24-step recurrence (rel err 0.15) while
prop-only fp8 stays at ~7e-3 (tolerance 2e-2).
"""
import sys

for _p in ("/opt/trn_rl_repo",):
    if _p not in sys.path:
        sys.path.insert(0, _p)

import numpy as np
import concourse.bass as bass
import concourse.mybir as mybir
import concourse.tile as tile
from concourse import bacc
from concourse.bass_utils import run_bass_kernel_spmd

fp32 = mybir.dt.float32
fp16 = mybir.dt.float16
fp8 = mybir.dt.float8e4
F8NP = mybir.dt.np(fp8)

NCORES = 8
B, T, NTOW, F = 512, 24, 10, 256
N = B * NTOW                  # 5120 nodes
NLOC = N // NCORES            # 640 nodes per core
NPASS = N // 256              # 20 DoubleRow contraction passes over nodes
KLOC = NLOC // 128            # 5 own-node 128-tiles (AG input shape)
FT = F // 128                 # 2 feature tiles
GM = (4 * F) // 128           # 8 gate-feature m-tiles
NOUT2 = 2 * NLOC              # 1280 = [U|V] output columns per core
S_H = 128.0                   # fp8 scale for H (|H| < 1 always)
LAMBDA_MAX = 2.0

NCH = [(0, 512), (512, 640)]             # node chunks for gate matmuls
PCH = [(0, 512), (512, 1024), (1024, 1280)]  # [U|V] column chunks
PORD = (2, 0, 1)  # issue order: small chunk first so last MM hides next LDW

SIG = mybir.ActivationFunctionType.Sigmoid
TANH = mybir.ActivationFunctionType.Tanh
COPY = mybir.ActivationFunctionType.Copy
DR = mybir.MatmulPerfMode.DoubleRow

_CACHE = {}


def _build_nc(repeat=1, no_comm=False):
    nc = bacc.Bacc(None, target_bir_lowering=False, num_devices=NCORES)
    d_ll2 = nc.dram_tensor("ll2", [NPASS, 128, 2, NOUT2], fp8, kind="ExternalInput")
    d_th = nc.dram_tensor("th", [GM, 128, 4 * F], fp16, kind="ExternalInput")
    d_x = nc.dram_tensor("xall", [T, FT, 128, NLOC], fp16, kind="ExternalInput")
    d_bias = nc.dram_tensor("biasv", [GM, 128], fp32, kind="ExternalInput")
    d_scl = nc.dram_tensor("sclv", [2, 128], fp32, kind="ExternalInput")
    d_h = nc.dram_tensor("hout", [FT, 128, NLOC], fp32, kind="ExternalOutput")
    d_c = nc.dram_tensor("cout", [FT, 128, NLOC], fp32, kind="ExternalOutput")

    with tile.TileContext(nc) as tc:
        with (
            tc.tile_pool(name="const", bufs=1) as constp,
            tc.tile_pool(name="xp", bufs=2) as xp,
            tc.tile_pool(name="gp", bufs=2) as gp,
            tc.tile_pool(name="uvp", bufs=1) as uvp,
            tc.tile_pool(name="hp", bufs=2) as hp,
            tc.tile_pool(name="hnmp", bufs=2) as hnmp,
            tc.tile_pool(name="tmpp", bufs=1) as tmpp,
            tc.tile_pool(name="psg", bufs=4, space="PSUM") as psg,
            tc.tile_pool(name="psp", bufs=4, space="PSUM") as psp,
            tc.tile_pool(name="dramio", bufs=2, space="DRAM") as dramp,
        ):
            # ---- resident tensors ----
            sb_ll2 = constp.tile([128, NPASS, 2, NOUT2], fp8, tag="ll2")
            sb_th = constp.tile([128, GM, 4 * F], fp16, tag="th")
            sb_bias = constp.tile([128, GM], fp32, tag="bias")
            sb_scl = constp.tile([128, 2], fp32, tag="scl")
            sb_hfull = constp.tile([128, NPASS, 2, F], fp8, tag="hfull")
            nc.sync.dma_start(sb_bias, d_bias.rearrange("m p -> p m"))
            nc.sync.dma_start(sb_scl, d_scl.rearrange("m p -> p m"))
            # theta in column chunks so step-0 gates can start early
            thv = d_th.rearrange("k p j -> p k j")
            for mc in range(GM):
                cs = slice(mc * 128, (mc + 1) * 128)
                nc.sync.dma_start(sb_th[:, :, cs], thv[:, :, cs])
            x_first = xp.tile([128, FT, NLOC], fp16, tag="x", name="x_first")
            nc.sync.dma_start(x_first, d_x[0].rearrange("f p n -> p f n"))
            for kg in range(NPASS // 5):
                ks = slice(kg * 5, (kg + 1) * 5)
                nc.sync.dma_start(
                    sb_ll2[:, ks], d_ll2[ks].rearrange("k p o j -> p k o j"))

            h_fm = None    # current H_i, feature-major [128, FT, NLOC] fp16
            c_fm = None    # current C_i, feature-major fp32

            first_iter = True
            for t in [tt for _r in range(repeat) for tt in range(T)]:
                if first_iter:
                    x_t = x_first
                    first_iter = False
                else:
                    x_t = xp.tile([128, FT, NLOC], fp16, tag="x", name=f"x{t}")
                    nc.sync.dma_start(x_t, d_x[t].rearrange("f p n -> p f n"))
                gacc = gp.tile([128, GM, NLOC], fp32, tag="g", name=f"g{t}")

                def rhs_of(kk, c0, c1, _x=x_t, _h=h_fm):
                    if kk < 2:
                        return _x[:, kk, c0:c1]
                    return _h[:, kk - 2, c0:c1]

                # ---- gate matmul, X(+H) part (fp16; AG-independent work) ----
                kks = (0, 1) if t == 0 else (0, 1, 2, 3)
                for m in range(GM):
                    pss = [
                        psg.tile([128, c1 - c0], fp32, tag="gps",
                                 name=f"gxh{t}_{m}_{ci}")
                        for ci, (c0, c1) in enumerate(NCH)
                    ]
                    for i, kk in enumerate(kks):
                        for ci, (c0, c1) in enumerate(NCH):
                            nc.tensor.matmul(
                                pss[ci],
                                sb_th[:, kk, m * 128:(m + 1) * 128],
                                rhs_of(kk, c0, c1),
                                start=(i == 0), stop=(i == len(kks) - 1))
                    for ci, (c0, c1) in enumerate(NCH):
                        nc.vector.tensor_copy(gacc[:, m, c0:c1], pss[ci])

                if t > 0:
                    # ---- mega-prop: fp8 DoubleRow, 20 x 256-deep passes ----
                    u_fm = uvp.tile([128, FT, NLOC], fp16, tag="u", name=f"u{t}")
                    v_fm = uvp.tile([128, FT, NLOC], fp16, tag="v", name=f"v{t}")
                    for m in range(FT):
                        ms = slice(m * 128, (m + 1) * 128)
                        pps = [
                            psp.tile([128, p1 - p0], fp32, tag="pps",
                                     name=f"pps{t}_{m}_{ci}")
                            for ci, (p0, p1) in enumerate(PCH)
                        ]
                        for k in range(NPASS):
                            lhsT = sb_hfull[:, k, :, ms]
                            for ci in PORD:
                                p0, p1 = PCH[ci]
                                nc.tensor.matmul(
                                    pps[ci], lhsT, sb_ll2[:, k, :, p0:p1],
                                    start=(k == 0), stop=(k == NPASS - 1),
                                    perf_mode=DR)
                        # descale U/V from PSUM to plain fp16 (scalar engine)
                        scl_u, scl_v = sb_scl[:, 0:1], sb_scl[:, 1:2]
                        nc.scalar.activation(u_fm[:, m, 0:512], pps[0], COPY,
                                             scale=scl_u)
                        nc.scalar.activation(u_fm[:, m, 512:640],
                                             pps[1][:, 0:128], COPY, scale=scl_u)
                        nc.scalar.activation(v_fm[:, m, 0:384],
                                             pps[1][:, 128:512], COPY, scale=scl_v)
                        nc.scalar.activation(v_fm[:, m, 384:640], pps[2], COPY,
                                             scale=scl_v)

                    # ---- gate matmul, U/V part (fp16, accumulate into gacc) ----
                    # even m-tiles first: they feed the ft=0 half of the LSTM,
                    # unblocking the first AllGather half earlier
                    for m in (0, 2, 4, 6, 1, 3, 5, 7):
                        pss = [
                            psg.tile([128, c1 - c0], fp32, tag="gps",
                                     name=f"guv{t}_{m}_{ci}")
                            for ci, (c0, c1) in enumerate(NCH)
                        ]
                        for i, kk in enumerate((4, 5, 6, 7)):
                            src = u_fm if kk < 6 else v_fm
                            for ci, (c0, c1) in enumerate(NCH):
                                nc.tensor.matmul(
                                    pss[ci],
                                    sb_th[:, kk, m * 128:(m + 1) * 128],
                                    src[:, kk % 2, c0:c1],
                                    start=(i == 0), stop=(i == 3))
                        for ci, (c0, c1) in enumerate(NCH):
                            nc.vector.tensor_add(
                                gacc[:, m, c0:c1], gacc[:, m, c0:c1], pss[ci])

                # ---- LSTM cell (feature-major, elementwise), then transpose
                # the fresh H slice, quantize to fp8, and kick the
                # feature-half AllGathers ----
                last = (t == T - 1)
                h_new = hp.tile([128, FT, NLOC], fp32 if last else fp16,
                                tag="h32" if last else "h", name=f"h{t + 1}",
                                bufs=1 if last else None)
                c_new = hp.tile([128, FT, NLOC], fp32, tag="c", name=f"c{t + 1}")
                if not last:
                    hnm = hnmp.tile([128, KLOC, F], fp16, tag="hnm",
                                    name=f"hnm{t}")
                    hnm8 = hnmp.tile([128, KLOC, F], fp8, tag="hnm8",
                                     name=f"hnm8{t}")
                    agins, agouts = [], []
                    for ft in range(FT):
                        agins.append(dramp.tile(
                            [NLOC, 128], fp8, tag=f"agin{ft}",
                            name=f"agin{t}_{ft}"))
                        agouts.append(dramp.tile(
                            [N, 128], fp8, tag=f"agout{ft}",
                            addr_space="Shared", name=f"agout{t}_{ft}"))

                def emit_ag(ft):
                    fs = slice(ft * 128, (ft + 1) * 128)
                    nc.sync.dma_start(
                        agins[ft].rearrange("(k p) f -> p k f", p=128),
                        hnm8[:, :, fs])
                    if not no_comm:
                        nc.gpsimd.collective_compute(
                            "AllGather",
                            mybir.AluOpType.bypass,
                            replica_groups=[list(range(NCORES))],
                            ins=[agins[ft].opt()],
                            outs=[agouts[ft].opt()],
                        )
                    agv = agouts[ft].rearrange("(k o p) f -> p k o f",
                                               p=128, o=2)
                    for kg in range(5):
                        ks = slice(kg * 4, (kg + 1) * 4)
                        nc.sync.dma_start(sb_hfull[:, ks, :, fs], agv[:, ks])
                for ft in range(FT):
                    ti = tmpp.tile([128, NLOC], fp16, tag="t1", name=f"ti{t}_{ft}")
                    tf = tmpp.tile([128, NLOC], fp16, tag="t2", name=f"tf{t}_{ft}")
                    tt = tmpp.tile([128, NLOC], fp16, tag="t3", name=f"tt{t}_{ft}")
                    to = tmpp.tile([128, NLOC], fp16, tag="t4", name=f"to{t}_{ft}")
                    tc2 = tmpp.tile([128, NLOC], fp16, tag="t1", name=f"tc{t}_{ft}")
                    nc.scalar.activation(ti, gacc[:, 0 + ft, :], SIG,
                                         bias=sb_bias[:, 0 + ft:1 + ft])
                    nc.scalar.activation(tf, gacc[:, 2 + ft, :], SIG,
                                         bias=sb_bias[:, 2 + ft:3 + ft])
                    nc.scalar.activation(tt, gacc[:, 4 + ft, :], TANH,
                                         bias=sb_bias[:, 4 + ft:5 + ft])
                    nc.scalar.activation(to, gacc[:, 6 + ft, :], SIG,
                                         bias=sb_bias[:, 6 + ft:7 + ft])
                    if t == 0:
                        nc.vector.tensor_mul(c_new[:, ft, :], ti, tt)
                    else:
                        nc.vector.tensor_mul(ti, ti, tt)
                        nc.vector.tensor_mul(tf, tf, c_fm[:, ft, :])
                        nc.vector.tensor_add(c_new[:, ft, :], ti, tf)
                    nc.scalar.activation(tc2, c_new[:, ft, :], TANH)
                    nc.vector.tensor_mul(h_new[:, ft, :], to, tc2)
                    if not last:
                        # node-major own slice (feature half ft), fp8-quantized
                        fs = slice(ft * 128, (ft + 1) * 128)
                        nc.sync.dma_start_transpose(hnm[:, :, fs],
                                                    h_new[:, ft, :])
                        nc.vector.tensor_scalar_mul(hnm8[:, :, fs],
                                                    hnm[:, :, fs], S_H)
                        emit_ag(ft)
                h_fm, c_fm = h_new, c_new

            nc.sync.dma_start(d_h.rearrange("f p n -> p f n"), h_fm)
            nc.sync.dma_start(d_c.rearrange("f p n -> p f n"), c_fm)

    nc.compile()
    return nc


def _q8(x, scale):
    return np.clip(np.asarray(x, np.float32) * scale,
                   -240.0, 240.0).astype(F8NP)


def _host_prep(X, edge_weight, Ws, bs, thetas, conv_bs, edge_index):
    """Build per-core device inputs from the raw problem inputs."""
    src = edge_index[0].astype(np.int64)
    dst = edge_index[1].astype(np.int64)
    ew = edge_weight.astype(np.float32)
    deg = np.bincount(src, weights=ew, minlength=N)
    dis = np.where(deg > 0, 1.0 / np.sqrt(np.where(deg > 0, deg, 1.0)), 0.0)
    dis = dis.astype(np.float32)
    w_hat = ((2.0 / LAMBDA_MAX) * (-dis[src] * ew * dis[dst])).astype(np.float32)
    diag = np.float32(2.0 / LAMBDA_MAX - 1.0)
    L = np.zeros((N, N), np.float32)
    np.add.at(L, (dst, src), w_hat)
    if diag != 0.0:
        L[np.arange(N), np.arange(N)] += diag
    L2 = L @ L

    # fp8 power-of-2 scales for the mega-prop operands
    s_L = 2.0 ** np.floor(np.log2(224.0 / max(np.abs(L).max(), 1e-30)))
    s_L2 = 2.0 ** np.floor(np.log2(224.0 / max(np.abs(L2).max(), 1e-30)))
    s_L = float(min(s_L, 2.0 ** 24))
    s_L2 = float(min(s_L2, 2.0 ** 24))
    scl = np.empty((2, 128), np.float32)
    scl[0] = 1.0 / (s_L * S_H)
    scl[1] = 1.0 / (s_L2 * S_H)

    # Theta: rows [X; H; U; V] x cols [I|F|T|O]
    Th = np.zeros((4 * F, 4 * F), np.float32)
    bias_full = np.zeros(4 * F, np.float32)
    for g in range(4):
        cs = slice(g * F, (g + 1) * F)
        Th[0 * F:1 * F, cs] = Ws[g]
        Th[1 * F:2 * F, cs] = thetas[g, 0] - thetas[g, 2]
        Th[2 * F:3 * F, cs] = thetas[g, 1]
        Th[3 * F:4 * F, cs] = 2.0 * thetas[g, 2]
        bias_full[cs] = bs[g] + conv_bs[g]
    th_t = np.ascontiguousarray(Th.reshape(GM, 128, 4 * F).astype(np.float16))
    bias_t = np.ascontiguousarray(bias_full.reshape(GM, 128).astype(np.float32))

    in_maps = []
    for i in range(NCORES):
        rows = slice(i * NLOC, (i + 1) * NLOC)
        rhs = np.concatenate([L[rows].T * s_L, L2[rows].T * s_L2], axis=1)
        # DoubleRow pairing: pass k contracts nodes (k*256 + o*128 + p)
        ll2 = np.ascontiguousarray(
            _q8(rhs, 1.0).reshape(NPASS, 2, 128, NOUT2).transpose(0, 2, 1, 3))
        # reference uses Xs = X.reshape(N, T, F) (torch-.view semantics: raw
        # memory reinterpretation), node n's time series is row n of that view
        xi = np.ascontiguousarray(
            X.reshape(N, T, F)[rows].transpose(1, 2, 0)
            .reshape(T, FT, 128, NLOC).astype(np.float16))
        in_maps.append(dict(ll2=ll2, th=th_t, xall=xi, biasv=bias_t, sclv=scl))
    return in_maps


def kernel(X, edge_weight, Ws, bs, thetas, conv_bs, edge_index):
    X = np.asarray(X, dtype=np.float32)
    edge_weight = np.asarray(edge_weight, dtype=np.float32)
    Ws = np.asarray(Ws, dtype=np.float32)
    bs = np.asarray(bs, dtype=np.float32)
    thetas = np.asarray(thetas, dtype=np.float32)
    conv_bs = np.asarray(conv_bs, dtype=np.float32)
    edge_index = np.asarray(edge_index)

    in_maps = _host_prep(X, edge_weight, Ws, bs, thetas, conv_bs, edge_index)
    if "nc" not in _CACHE:
        _CACHE["nc"] = _build_nc()
    nc = _CACHE["nc"]
    res = run_bass_kernel_spmd(nc, in_maps, core_ids=list(range(NCORES)))

    H = np.empty((N, F), np.float32)
    C = np.empty((N, F), np.float32)
    for i in range(NCORES):
        rows = slice(i * NLOC, (i + 1) * NLOC)
        H[rows] = res.results[i]["hout"].reshape(F, NLOC).T
        C[rows] = res.results[i]["cout"].reshape(F, NLOC).T
    return H, C


# revision 26
# speedup vs baseline: 7.5617x; 7.5617x over previous
"""GCLSTM (ChebConv-gated LSTM) Trainium2 kernel, 8-core SPMD, fp8 mega-prop.

Algorithm notes
---------------
reference computes, per timestep t (T=24) over N=5120 graph nodes:
    gate_g = X_t @ Ws[g] + cheb(H, thetas[g]) + biases      (4 gates)
    cheb(H, th) = H@th0 + (L@H)@th1 + (2L(LH) - H)@th2      (K=3 Chebyshev)
with L the scaled-normalized graph Laplacian (5120x5120, sparse, here
densified).  The Chebyshev basis (U = L@H, V = L^2@H) is shared by all 4
gates, so per step we need exactly ONE dense "mega-prop" [U|V] = [L;L^2]@H
plus the gate matmuls.  Folding:
    gate_g = X_t@Ws[g] + H@(th0-th2) + U@th1 + V@(2*th2) + b
so all gate work is a single [X;H;U;V] (1024) x Theta (1024x1024) matmul.

Sharding: nodes are split across 8 cores (640 each; edges connect
arbitrary nodes, so each core holds the full [L;L^2] column block for its
output rows, resident in SBUF).  The mega-prop contracts over ALL
5120 nodes, so the full H (node-major) is re-assembled every step with
two feature-half AllGathers.

Precision: the mega-prop runs in fp8 e4m3 with perf_mode=DoubleRow
(256-deep contraction, 2x PE throughput): ll2 holds [s_L*L | s_L2*L^2]
quantized to fp8, H is quantized to s_h*H fp8 (s_h=128) right after the
node-major transpose, and the AllGather moves fp8 bytes (half the wire
traffic).  U/V are descaled back to plain fp16 during the PSUM->SBUF
copy (scalar engine, scale from a tiny input tensor).  The gate matmul,
LSTM cell, and all theta weights stay fp16/fp32: simulation shows gate-
side fp8 compounds through the 24-step recurrence (rel err 0.15) while
prop-only fp8 stays at ~7e-3 (tolerance 2e-2).
"""
import sys

for _p in ("/opt/trn_rl_repo",):
    if _p not in sys.path:
        sys.path.insert(0, _p)

import numpy as np
import concourse.bass as bass
import concourse.mybir as mybir
import concourse.tile as tile
from concourse import bacc
from concourse.bass_utils import run_bass_kernel_spmd

fp32 = mybir.dt.float32
fp16 = mybir.dt.float16
fp8 = mybir.dt.float8e4
F8NP = mybir.dt.np(fp8)

NCORES = 8
B, T, NTOW, F = 512, 24, 10, 256
N = B * NTOW                  # 5120 nodes
NLOC = N // NCORES            # 640 nodes per core
NPASS = N // 256              # 20 DoubleRow contraction passes over nodes
KLOC = NLOC // 128            # 5 own-node 128-tiles (AG input shape)
FT = F // 128                 # 2 feature tiles
GM = (4 * F) // 128           # 8 gate-feature m-tiles
NOUT2 = 2 * NLOC              # 1280 = [U|V] output columns per core
S_H = 128.0                   # fp8 scale for H (|H| < 1 always)
LAMBDA_MAX = 2.0

NCH = [(0, 512), (512, 640)]             # node chunks for gate matmuls
PCH = [(0, 512), (512, 1024), (1024, 1280)]  # [U|V] column chunks
PORD = (2, 0, 1)  # issue order: small chunk first so last MM hides next LDW

SIG = mybir.ActivationFunctionType.Sigmoid
TANH = mybir.ActivationFunctionType.Tanh
COPY = mybir.ActivationFunctionType.Copy
DR = mybir.MatmulPerfMode.DoubleRow

_CACHE = {}


def _build_nc(repeat=1, no_comm=False, ag_bc16=False, fake_ag=False,
              n_dummy=0, ag_unused=False, hfull_q_act=False):
    nc = bacc.Bacc(None, target_bir_lowering=False, num_devices=NCORES)
    d_ll2 = nc.dram_tensor("ll2", [NPASS, 128, 2, NOUT2], fp8, kind="ExternalInput")
    d_th = nc.dram_tensor("th", [GM, 128, 4 * F], fp16, kind="ExternalInput")
    d_x = nc.dram_tensor("xall", [T, FT, 128, NLOC], fp16, kind="ExternalInput")
    d_bias = nc.dram_tensor("biasv", [GM, 128], fp32, kind="ExternalInput")
    d_scl = nc.dram_tensor("sclv", [2, 128], fp32, kind="ExternalInput")
    d_h = nc.dram_tensor("hout", [FT, 128, NLOC], fp32, kind="ExternalOutput")
    d_c = nc.dram_tensor("cout", [FT, 128, NLOC], fp32, kind="ExternalOutput")

    with tile.TileContext(nc) as tc:
        with (
            tc.tile_pool(name="const", bufs=1) as constp,
            tc.tile_pool(name="xp", bufs=2) as xp,
            tc.tile_pool(name="gp", bufs=2) as gp,
            tc.tile_pool(name="uvp", bufs=1) as uvp,
            tc.tile_pool(name="hp", bufs=2) as hp,
            tc.tile_pool(name="hnmp", bufs=2) as hnmp,
            tc.tile_pool(name="tmpp", bufs=1) as tmpp,
            tc.tile_pool(name="psg", bufs=4, space="PSUM") as psg,
            tc.tile_pool(name="psp", bufs=4, space="PSUM") as psp,
            tc.tile_pool(name="dramio", bufs=2, space="DRAM") as dramp,
        ):
            # ---- resident tensors ----
            sb_ll2 = constp.tile([128, NPASS, 2, NOUT2], fp8, tag="ll2")
            sb_th = constp.tile([128, GM, 4 * F], fp16, tag="th")
            sb_bias = constp.tile([128, GM], fp32, tag="bias")
            sb_scl = constp.tile([128, 2], fp32, tag="scl")
            sb_hfull = constp.tile([128, NPASS, 2, F], fp8, tag="hfull")
            if ag_unused:
                # timing probe: prop reads this const tile; AG output lands
                # in sb_hfull which nobody reads (AG off the dep chain)
                sb_hconst = constp.tile([128, NPASS, 2, F], fp8, tag="hconst")
                nc.vector.memset(sb_hconst, 0)
            nc.sync.dma_start(sb_bias, d_bias.rearrange("m p -> p m"))
            nc.sync.dma_start(sb_scl, d_scl.rearrange("m p -> p m"))
            # theta in column chunks so step-0 gates can start early
            thv = d_th.rearrange("k p j -> p k j")
            for mc in range(GM):
                cs = slice(mc * 128, (mc + 1) * 128)
                nc.sync.dma_start(sb_th[:, :, cs], thv[:, :, cs])
            x_first = xp.tile([128, FT, NLOC], fp16, tag="x", name="x_first")
            nc.sync.dma_start(x_first, d_x[0].rearrange("f p n -> p f n"))
            for kg in range(NPASS // 5):
                ks = slice(kg * 5, (kg + 1) * 5)
                nc.sync.dma_start(
                    sb_ll2[:, ks], d_ll2[ks].rearrange("k p o j -> p k o j"))

            h_fm = None    # current H_i, feature-major [128, FT, NLOC] fp16
            c_fm = None    # current C_i, feature-major fp32

            if n_dummy:
                dmy_in = dramp.tile([128, 8], fp16, tag="dmyi", name="dmyi",
                                    bufs=1)
                nc.sync.dma_start(
                    dmy_in.rearrange("(k p) f -> p k f", p=128),
                    sb_bias[:, 0:8].bitcast(fp16)[:, 0:8])
                dmy_ctr = [0]

            def emit_dummies():
                for _ in range(n_dummy):
                    i = dmy_ctr[0] % 4
                    dmy_ctr[0] += 1
                    dmy_out = dramp.tile(
                        [1024, 8], fp16, tag=f"dmyo{i}",
                        addr_space="Shared", name=f"dmyo{dmy_ctr[0]}")
                    nc.gpsimd.collective_compute(
                        "AllGather", mybir.AluOpType.bypass,
                        replica_groups=[list(range(NCORES))],
                        ins=[dmy_in.opt()], outs=[dmy_out.opt()])

            first_iter = True
            for t in [tt for _r in range(repeat) for tt in range(T)]:
                if first_iter:
                    x_t = x_first
                    first_iter = False
                else:
                    x_t = xp.tile([128, FT, NLOC], fp16, tag="x", name=f"x{t}")
                    nc.sync.dma_start(x_t, d_x[t].rearrange("f p n -> p f n"))
                gacc = gp.tile([128, GM, NLOC], fp32, tag="g", name=f"g{t}")

                def rhs_of(kk, c0, c1, _x=x_t, _h=h_fm):
                    if kk < 2:
                        return _x[:, kk, c0:c1]
                    return _h[:, kk - 2, c0:c1]

                # ---- gate matmul, X(+H) part (fp16; AG-independent work) ----
                kks = (0, 1) if t == 0 else (0, 1, 2, 3)
                for m in range(GM):
                    pss = [
                        psg.tile([128, c1 - c0], fp32, tag="gps",
                                 name=f"gxh{t}_{m}_{ci}")
                        for ci, (c0, c1) in enumerate(NCH)
                    ]
                    for i, kk in enumerate(kks):
                        for ci, (c0, c1) in enumerate(NCH):
                            nc.tensor.matmul(
                                pss[ci],
                                sb_th[:, kk, m * 128:(m + 1) * 128],
                                rhs_of(kk, c0, c1),
                                start=(i == 0), stop=(i == len(kks) - 1))
                    for ci, (c0, c1) in enumerate(NCH):
                        nc.vector.tensor_copy(gacc[:, m, c0:c1], pss[ci])

                if t > 0:
                    # ---- mega-prop: fp8 DoubleRow, 20 x 256-deep passes ----
                    u_fm = uvp.tile([128, FT, NLOC], fp16, tag="u", name=f"u{t}")
                    v_fm = uvp.tile([128, FT, NLOC], fp16, tag="v", name=f"v{t}")
                    for m in range(FT):
                        ms = slice(m * 128, (m + 1) * 128)
                        pps = [
                            psp.tile([128, p1 - p0], fp32, tag="pps",
                                     name=f"pps{t}_{m}_{ci}")
                            for ci, (p0, p1) in enumerate(PCH)
                        ]
                        for k in range(NPASS):
                            lhsT = (sb_hconst if ag_unused
                                    else sb_hfull)[:, k, :, ms]
                            for ci in PORD:
                                p0, p1 = PCH[ci]
                                nc.tensor.matmul(
                                    pps[ci], lhsT, sb_ll2[:, k, :, p0:p1],
                                    start=(k == 0), stop=(k == NPASS - 1),
                                    perf_mode=DR)
                        # descale U/V from PSUM to plain fp16 (scalar engine)
                        scl_u, scl_v = sb_scl[:, 0:1], sb_scl[:, 1:2]
                        nc.scalar.activation(u_fm[:, m, 0:512], pps[0], COPY,
                                             scale=scl_u)
                        nc.scalar.activation(u_fm[:, m, 512:640],
                                             pps[1][:, 0:128], COPY, scale=scl_u)
                        nc.scalar.activation(v_fm[:, m, 0:384],
                                             pps[1][:, 128:512], COPY, scale=scl_v)
                        nc.scalar.activation(v_fm[:, m, 384:640], pps[2], COPY,
                                             scale=scl_v)

                    # ---- gate matmul, U/V part (fp16, accumulate into gacc) ----
                    # even m-tiles first: they feed the ft=0 half of the LSTM,
                    # unblocking the first AllGather half earlier
                    for m in (0, 2, 4, 6, 1, 3, 5, 7):
                        pss = [
                            psg.tile([128, c1 - c0], fp32, tag="gps",
                                     name=f"guv{t}_{m}_{ci}")
                            for ci, (c0, c1) in enumerate(NCH)
                        ]
                        for i, kk in enumerate((4, 5, 6, 7)):
                            src = u_fm if kk < 6 else v_fm
                            for ci, (c0, c1) in enumerate(NCH):
                                nc.tensor.matmul(
                                    pss[ci],
                                    sb_th[:, kk, m * 128:(m + 1) * 128],
                                    src[:, kk % 2, c0:c1],
                                    start=(i == 0), stop=(i == 3))
                        for ci, (c0, c1) in enumerate(NCH):
                            nc.vector.tensor_add(
                                gacc[:, m, c0:c1], gacc[:, m, c0:c1], pss[ci])

                # ---- LSTM cell (feature-major, elementwise), then transpose
                # the fresh H slice, quantize to fp8, and kick the
                # feature-half AllGathers ----
                last = (t == T - 1)
                h_new = hp.tile([128, FT, NLOC], fp32 if last else fp16,
                                tag="h32" if last else "h", name=f"h{t + 1}",
                                bufs=1 if last else None)
                c_new = hp.tile([128, FT, NLOC], fp32, tag="c", name=f"c{t + 1}")
                if not last:
                    hnm = hnmp.tile([128, KLOC, F], fp16, tag="hnm",
                                    name=f"hnm{t}")
                    hnm8 = hnmp.tile([128, KLOC, F], fp8, tag="hnm8",
                                     name=f"hnm8{t}")
                    agins, agouts = [], []
                    agdt, agw = (fp16, 64) if ag_bc16 else (fp8, 128)
                    for ft in range(FT):
                        agins.append(dramp.tile(
                            [NLOC, agw], agdt, tag=f"agin{ft}",
                            name=f"agin{t}_{ft}"))
                        agouts.append(dramp.tile(
                            [N, agw], agdt, tag=f"agout{ft}",
                            addr_space="Shared", name=f"agout{t}_{ft}"))

                def emit_ag(ft):
                    # per-feature-half AllGather (fp8 payload shipped in
                    # fp16-typed buffers via bitcast): ft=0 launches right
                    # after the first cell half, so the two collectives
                    # pipeline and prop m=0 unblocks earliest
                    fs = slice(ft * 128, (ft + 1) * 128)
                    agi = agins[ft].bitcast(fp8) if ag_bc16 else agins[ft]
                    nc.sync.dma_start(
                        agi.rearrange("(k p) f -> p k f", p=128),
                        hnm8[:, :, fs])
                    if not no_comm:
                        nc.gpsimd.collective_compute(
                            "AllGather",
                            mybir.AluOpType.bypass,
                            replica_groups=[list(range(NCORES))],
                            ins=[agins[ft].opt()],
                            outs=[agouts[ft].opt()],
                        )
                    ago = agouts[ft].bitcast(fp8) if ag_bc16 else agouts[ft]
                    agv = ago.rearrange("(k o p) f -> p k o f", p=128, o=2)
                    # hfull-in is the AG consumer (critical path): issue from
                    # the Activation HWDGE queue to dodge SP head-of-line
                    eng = nc.scalar if hfull_q_act else nc.sync
                    for kg in range(5):
                        ks = slice(kg * 4, (kg + 1) * 4)
                        eng.dma_start(sb_hfull[:, ks, :, fs], agv[:, ks])
                for ft in range(FT):
                    ti = tmpp.tile([128, NLOC], fp16, tag="t1", name=f"ti{t}_{ft}")
                    tf = tmpp.tile([128, NLOC], fp16, tag="t2", name=f"tf{t}_{ft}")
                    tt = tmpp.tile([128, NLOC], fp16, tag="t3", name=f"tt{t}_{ft}")
                    to = tmpp.tile([128, NLOC], fp16, tag="t4", name=f"to{t}_{ft}")
                    tc2 = tmpp.tile([128, NLOC], fp16, tag="t1", name=f"tc{t}_{ft}")
                    nc.scalar.activation(ti, gacc[:, 0 + ft, :], SIG,
                                         bias=sb_bias[:, 0 + ft:1 + ft])
                    nc.scalar.activation(tf, gacc[:, 2 + ft, :], SIG,
                                         bias=sb_bias[:, 2 + ft:3 + ft])
                    nc.scalar.activation(tt, gacc[:, 4 + ft, :], TANH,
                                         bias=sb_bias[:, 4 + ft:5 + ft])
                    nc.scalar.activation(to, gacc[:, 6 + ft, :], SIG,
                                         bias=sb_bias[:, 6 + ft:7 + ft])
                    if t == 0:
                        nc.vector.tensor_mul(c_new[:, ft, :], ti, tt)
                    else:
                        nc.vector.tensor_mul(ti, ti, tt)
                        nc.vector.tensor_mul(tf, tf, c_fm[:, ft, :])
                        nc.vector.tensor_add(c_new[:, ft, :], ti, tf)
                    nc.scalar.activation(tc2, c_new[:, ft, :], TANH)
                    nc.vector.tensor_mul(h_new[:, ft, :], to, tc2)
                    if not last:
                        # node-major own slice (feature half ft), fp8-quantized
                        fs = slice(ft * 128, (ft + 1) * 128)
                        nc.sync.dma_start_transpose(hnm[:, :, fs],
                                                    h_new[:, ft, :])
                        nc.vector.tensor_scalar_mul(hnm8[:, :, fs],
                                                    hnm[:, :, fs], S_H)
                        emit_ag(ft)
                if n_dummy and not last:
                    emit_dummies()
                h_fm, c_fm = h_new, c_new

            nc.sync.dma_start(d_h.rearrange("f p n -> p f n"), h_fm)
            nc.sync.dma_start(d_c.rearrange("f p n -> p f n"), c_fm)

    nc.compile()
    return nc


def _q8(x, scale):
    return np.clip(np.asarray(x, np.float32) * scale,
                   -240.0, 240.0).astype(F8NP)


def _host_prep(X, edge_weight, Ws, bs, thetas, conv_bs, edge_index):
    """Build per-core device inputs from the raw problem inputs."""
    src = edge_index[0].astype(np.int64)
    dst = edge_index[1].astype(np.int64)
    ew = edge_weight.astype(np.float32)
    deg = np.bincount(src, weights=ew, minlength=N)
    dis = np.where(deg > 0, 1.0 / np.sqrt(np.where(deg > 0, deg, 1.0)), 0.0)
    dis = dis.astype(np.float32)
    w_hat = ((2.0 / LAMBDA_MAX) * (-dis[src] * ew * dis[dst])).astype(np.float32)
    diag = np.float32(2.0 / LAMBDA_MAX - 1.0)
    L = np.zeros((N, N), np.float32)
    np.add.at(L, (dst, src), w_hat)
    if diag != 0.0:
        L[np.arange(N), np.arange(N)] += diag
    L2 = L @ L

    # fp8 power-of-2 scales for the mega-prop operands
    s_L = 2.0 ** np.floor(np.log2(224.0 / max(np.abs(L).max(), 1e-30)))
    s_L2 = 2.0 ** np.floor(np.log2(224.0 / max(np.abs(L2).max(), 1e-30)))
    s_L = float(min(s_L, 2.0 ** 24))
    s_L2 = float(min(s_L2, 2.0 ** 24))
    scl = np.empty((2, 128), np.float32)
    scl[0] = 1.0 / (s_L * S_H)
    scl[1] = 1.0 / (s_L2 * S_H)

    # Theta: rows [X; H; U; V] x cols [I|F|T|O]
    Th = np.zeros((4 * F, 4 * F), np.float32)
    bias_full = np.zeros(4 * F, np.float32)
    for g in range(4):
        cs = slice(g * F, (g + 1) * F)
        Th[0 * F:1 * F, cs] = Ws[g]
        Th[1 * F:2 * F, cs] = thetas[g, 0] - thetas[g, 2]
        Th[2 * F:3 * F, cs] = thetas[g, 1]
        Th[3 * F:4 * F, cs] = 2.0 * thetas[g, 2]
        bias_full[cs] = bs[g] + conv_bs[g]
    th_t = np.ascontiguousarray(Th.reshape(GM, 128, 4 * F).astype(np.float16))
    bias_t = np.ascontiguousarray(bias_full.reshape(GM, 128).astype(np.float32))

    in_maps = []
    for i in range(NCORES):
        rows = slice(i * NLOC, (i + 1) * NLOC)
        rhs = np.concatenate([L[rows].T * s_L, L2[rows].T * s_L2], axis=1)
        # DoubleRow pairing: pass k contracts nodes (k*256 + o*128 + p)
        ll2 = np.ascontiguousarray(
            _q8(rhs, 1.0).reshape(NPASS, 2, 128, NOUT2).transpose(0, 2, 1, 3))
        # reference uses Xs = X.reshape(N, T, F) (torch-.view semantics: raw
        # memory reinterpretation), node n's time series is row n of that view
        xi = np.ascontiguousarray(
            X.reshape(N, T, F)[rows].transpose(1, 2, 0)
            .reshape(T, FT, 128, NLOC).astype(np.float16))
        in_maps.append(dict(ll2=ll2, th=th_t, xall=xi, biasv=bias_t, sclv=scl))
    return in_maps


def kernel(X, edge_weight, Ws, bs, thetas, conv_bs, edge_index):
    X = np.asarray(X, dtype=np.float32)
    edge_weight = np.asarray(edge_weight, dtype=np.float32)
    Ws = np.asarray(Ws, dtype=np.float32)
    bs = np.asarray(bs, dtype=np.float32)
    thetas = np.asarray(thetas, dtype=np.float32)
    conv_bs = np.asarray(conv_bs, dtype=np.float32)
    edge_index = np.asarray(edge_index)

    in_maps = _host_prep(X, edge_weight, Ws, bs, thetas, conv_bs, edge_index)
    if "nc" not in _CACHE:
        _CACHE["nc"] = _build_nc()
    nc = _CACHE["nc"]
    res = run_bass_kernel_spmd(nc, in_maps, core_ids=list(range(NCORES)))

    H = np.empty((N, F), np.float32)
    C = np.empty((N, F), np.float32)
    for i in range(NCORES):
        rows = slice(i * NLOC, (i + 1) * NLOC)
        H[rows] = res.results[i]["hout"].reshape(F, NLOC).T
        C[rows] = res.results[i]["cout"].reshape(F, NLOC).T
    return H, C
